# revision 2
# baseline (speedup 1.0000x reference)
"""Bahdanau (additive MLP) attention on 8 Trainium2 NeuronCores — v3.

Approximation: tanh(x) ~= a1 sin(w1 x) + a3 sin(3 w1 x) + a6 sin(6 w1 x),
w1 = pi/L, coefficients polished against an exact bf16 plane-algebra
emulation of this kernel (end-to-end rel err 0.0121 vs the f32 reference).

aligns[m,n] = sum_h w~[h] tanh(qp[m,h] + mp[n,h]) expands via
sin(f(q+m)) = sin_f(q)cos_f(m) + cos_f(q)sin_f(m) into 6 PE pair-matmuls
with contraction (h). f6 planes derive from f3 by angle doubling:
  q side: sg6q = s3q*c3q (=sin6/2), t6q = s3q^2, c6q = 1-2 t6q (true cos)
  m side: t6m = s3m^2 (=(1-cos6)/2), sg6m = s3m*c3m (=sin6/2)
  a6[s6q c6m + c6q s6m] = 2 a6 sg6q (per-m const, drops in softmax)
                          - 4 a6 sg6q t6m + 2 a6 c6q sg6m
Pairs (q-plane folded with alpha*w~ masks, m-plane raw):
  (a1 w~ s1q, c1m) (a1 w~ c1q, s1m) (a3 w~ s3q, c3m) (a3 w~ c3q, s3m)
  (-4 a6 w~ sg6q, t6m) (2 a6 w~ c6q, sg6m)

Aligns are accumulated TRANSPOSED: alT[n-chunk][np, m] via stationary
m-plane slices, so softmax rowsums and the output matmul contract n on
partitions with no score transposes: sums = expT.T @ ones, out =
expT.T @ memN, both plain PE matmuls. exp runs per 128-wide n-chunk so
the output matmul pipelines behind it.

Schedule: DMAs ordered qT, wqT, wm/mem interleaved halves (single DMA
engine in the cost model is serial; q tensors first so the four q-side
sins pack the ACT engine while m-proj finishes), then 4 m-side sins
(f3 pair first: their DVE doublings feed feature matmuls immediately;
f1 pair last gates only its own 32 matmuls). No PE warmup/fillers: the
TimelineSim p-state ramp depends only on elapsed time, and all matmuls
here start after the 3us warm threshold.

Sharding: core i = batch i//2, M-half i%2 (128 query rows). Fully data
parallel, softmax local, no collectives.
"""

import numpy as np

import concourse.tile as tile
from concourse import bacc, mybir
from concourse.alu_op_type import AluOpType
from concourse.bass_utils import run_bass_kernel_spmd

f32 = mybir.dt.float32
bf16 = mybir.dt.bfloat16
AF = mybir.ActivationFunctionType
MULT = AluOpType.mult
ADD = AluOpType.add

B, M, N, D, H = 4, 256, 512, 512, 512
NCORES = 8
ML = M * B // NCORES  # 128 query rows per core

# tanh(x) ~= sum a_f sin(f*w1*x), f in {1,3,6}; polished fit (see docstring)
A1, A3, A6 = 1.17491041, 0.31016341, 0.08701841
L_RANGE = 8.58851971
W1 = np.pi / L_RANGE

# q-side fold masks: mask_u[h] = alpha_u * w~[h]
MASKS = (("1", A1), ("3", A3), ("6s", -4 * A6), ("6c", 2 * A6))
MIDX = {name: i for i, (name, _) in enumerate(MASKS)}


def _build():
    nc = bacc.Bacc("TRN2", target_bir_lowering=False, debug=False, num_devices=NCORES)

    qT = nc.dram_tensor("qT", [128, 512], bf16, kind="ExternalInput")
    wqT = nc.dram_tensor("wqT", [128, 2048], bf16, kind="ExternalInput")
    wmT = nc.dram_tensor("wmT", [128, 2048], bf16, kind="ExternalInput")
    memT = nc.dram_tensor("memT", [128, 2048], bf16, kind="ExternalInput")
    memN = nc.dram_tensor("memN", [128, 2048], bf16, kind="ExternalInput")
    msk = nc.dram_tensor("msk", [128, len(MASKS) * 4], f32, kind="ExternalInput")
    out = nc.dram_tensor("out", [128, 512], bf16, kind="ExternalOutput")
    sums = nc.dram_tensor("sums", [128, 1], f32, kind="ExternalOutput")

    with tile.TileContext(nc) as tc:
        with (
            tc.tile_pool(name="const", bufs=1) as const,
            tc.tile_pool(name="mpp", bufs=1, space="PSUM") as mpp,
            tc.tile_pool(name="qp_pool", bufs=1, space="PSUM") as qp_pool,
            tc.tile_pool(name="al_pool", bufs=1, space="PSUM") as al_pool,
        ):
            # ---- SBUF tiles ----------------------------------------------
            warm_sb = const.tile([128, 128], bf16)
            zeros_sb = const.tile([128, 512], bf16)
            qT_sb = const.tile([128, 512], bf16)
            wqT_sb = const.tile([128, 2048], bf16)
            wmT_sb = const.tile([128, 2048], bf16)
            memT_sb = const.tile([128, 2048], bf16)
            memN_sb = const.tile([128, 2048], bf16)
            msk_sb = const.tile([128, len(MASKS) * 4], f32)
            ones_sb = const.tile([128, 1], bf16)
            halfpi_sb = const.tile([128, 1], f32)

            qs1 = const.tile([128, 512], bf16, name="qs1")
            qc1 = const.tile([128, 512], bf16, name="qc1")
            qs3 = const.tile([128, 512], bf16, name="qs3")
            qc3 = const.tile([128, 512], bf16, name="qc3")
            qsh3 = const.tile([128, 512], bf16, name="qsh3")
            qth3 = const.tile([128, 512], bf16, name="qth3")
            t6q = const.tile([128, 512], bf16, name="t6q")
            sg6q = const.tile([128, 512], bf16, name="sg6q")
            c6q = const.tile([128, 512], bf16, name="c6q")
            ms3 = const.tile([128, 2048], bf16, name="ms3")
            mc3 = const.tile([128, 2048], bf16, name="mc3")
            msh3 = const.tile([128, 2048], bf16, name="msh3")
            mth3 = const.tile([128, 2048], bf16, name="mth3")
            ms1 = const.tile([128, 2048], bf16, name="ms1")
            mc1 = const.tile([128, 2048], bf16, name="mc1")
            t6m = const.tile([128, 2048], bf16, name="t6m")
            sg6m = const.tile([128, 2048], bf16, name="sg6m")
            A = {k: const.tile([128, 512], bf16, name=f"A{k}")
                 for k in ("1s", "1c", "3s", "3c", "6s", "6c")}
            expT_sb = const.tile([128, 512], bf16, name="expT")
            out_sb = const.tile([128, 512], bf16)
            sums_sb = const.tile([128, 1], f32)

            # ---- PSUM tiles ----------------------------------------------
            mp = mpp.tile([128, 2048], f32, tag="mp", name="mp")
            qp = qp_pool.tile([128, 512], f32, tag="qp", name="qp")
            alT = al_pool.tile([128, 512], f32, tag="al", name="alT")
            # two separate output PSUM tiles so the ACT and DVE copies at
            # the tail overlap (one shared tile serializes them)
            out_ps0 = qp_pool.tile([128, 256], f32, tag="op0", name="out_ps0")
            out_ps1 = qp_pool.tile([128, 256], f32, tag="op1", name="out_ps1")
            # reuses qp's bank (qp is dead after the q-side activations)
            sums_ps = qp_pool.tile([128, 1], f32, tag="qp", name="sums_ps")

            # ---- prologue ------------------------------------------------
            nc.vector.memset(ones_sb[:], 1.0)
            nc.vector.memset(halfpi_sb[:], float(np.pi / 2))
            nc.vector.memset(warm_sb[:], 1.0)
            nc.vector.memset(zeros_sb[:], 0.0)

            # DMA order (single serial DMA engine in the cost model):
            # q tensors first, then wm/mem interleaved halves, memN last.
            # big wqT first: q-proj is gated by max(qT, wqT) arrival, so the
            # small qT rides behind without delaying it
            nc.sync.dma_start(wqT_sb[:], wqT.ap())
            nc.sync.dma_start(qT_sb[:], qT.ap())
            # wm/mem slices sized (1024,512,512): the big slice lands while
            # q-proj runs; the small trailing slices keep the last m-proj
            # group's start early without dispatch-queue serialization.
            for sl in (slice(0, 1024), slice(1024, 1536), slice(1536, 2048)):
                nc.sync.dma_start(wmT_sb[:, sl], wmT.ap()[:, sl])
                nc.sync.dma_start(memT_sb[:, sl], memT.ap()[:, sl])
            nc.sync.dma_start(memN_sb[:], memN.ap())
            nc.gpsimd.dma_start(msk_sb[:], msk.ap())

            # PE warmups: TimelineSim's p-state clock starts at the first
            # matmul; issue cheap ones immediately so the real matmuls
            # (first at ~4.7us) run at full rate. out_ps is overwritten
            # later by an accumulation group with start=True.
            # PE warmups that double as zero-clears of the accumulation
            # targets (writes 0 and sets has_written), so every later
            # matmul accumulates with start=False in ANY schedule order --
            # interleaved start=True groups get clobbered when the
            # scheduler reorders them. All warmups write zeros, so their
            # order relative to each other is irrelevant too.
            for _ in range(3):
                nc.tensor.matmul(alT[:], warm_sb[:], zeros_sb[:],
                                 start=True, stop=True)
            nc.tensor.matmul(out_ps0[:], warm_sb[:], zeros_sb[:, 0:256],
                             start=True, stop=True)
            nc.tensor.matmul(out_ps1[:], warm_sb[:], zeros_sb[:, 0:256],
                             start=True, stop=True)
            # NOTE: no zero-clear for sums_ps -- it aliases qp's bank, and
            # an early PE-FIFO clear waiting on qp's death deadlocks the
            # queue. Its 4-matmul group below keeps start=True instead
            # (contiguous, uniform readiness).

            # ---- projections ---------------------------------------------
            # q-proj: qp[hp, (c,m)] = Wq~.T @ query.T   (c-outer, dc-inner)
            for c in range(4):
                for dc in range(4):
                    nc.tensor.matmul(
                        qp[:, c * 128:(c + 1) * 128],
                        wqT_sb[:, dc * 512 + c * 128: dc * 512 + (c + 1) * 128],
                        qT_sb[:, dc * 128:(dc + 1) * 128],
                        start=(dc == 0), stop=(dc == 3),
                    )
            # m-proj: mp[hp, (c,n)] (dc-outer so dc-halves consume the
            # wm/mem DMA halves as they land)
            for dc in range(4):
                for c in range(4):
                    nc.tensor.matmul(
                        mp[:, c * 512:(c + 1) * 512],
                        wmT_sb[:, dc * 512 + c * 128: dc * 512 + (c + 1) * 128],
                        memT_sb[:, dc * 512:(dc + 1) * 512],
                        start=(dc == 0), stop=(dc == 3),
                    )

            # ---- q-side planes + folds -----------------------------------
            def act(dst, src, w, cos=False):
                if cos:
                    nc.scalar.activation(dst[:], src[:], AF.Sin,
                                         bias=halfpi_sb[:], scale=float(w))
                else:
                    nc.scalar.activation(dst[:], src[:], AF.Sin, scale=float(w))

            # f3 cos via half-angle: cos(3w1 x) = 1 - 2 sin^2(1.5 w1 x).
            # The direct form sin(3w1 x + pi/2) sends arguments past 2*pi
            # where the device Sin table degrades (the sin planes stay under
            # 2*pi and are computed directly).
            act(qs1, qp, W1)
            act(qc1, qp, W1, cos=True)
            act(qs3, qp, 3 * W1)
            act(qsh3, qp, 1.5 * W1)

            tt = nc.vector.tensor_tensor
            ts = nc.vector.tensor_scalar

            tt(qth3[:], qsh3[:], qsh3[:], MULT)
            ts(qc3[:], qth3[:], -2.0, 1.0, MULT, ADD)
            tt(t6q[:], qs3[:], qs3[:], MULT)
            tt(sg6q[:], qs3[:], qc3[:], MULT)
            ts(c6q[:], t6q[:], -2.0, 1.0, MULT, ADD)

            def fold(dst, srcp, name, eng):
                u = MIDX[name]
                for c in range(4):
                    eng.tensor_scalar_mul(
                        dst[:, c * 128:(c + 1) * 128],
                        srcp[:, c * 128:(c + 1) * 128],
                        msk_sb[:, u * 4 + c: u * 4 + c + 1],
                    )

            # f1 folds on gpsimd (idle engine), rest on DVE
            fold(A["1s"], qs1, "1", nc.gpsimd)
            fold(A["1c"], qc1, "1", nc.gpsimd)
            fold(A["3s"], qs3, "3", nc.vector)
            fold(A["3c"], qc3, "3", nc.vector)
            fold(A["6s"], sg6q, "6s", nc.vector)
            fold(A["6c"], c6q, "6c", nc.vector)

            # ---- m-side planes -------------------------------------------
            # f3 pair first: feeds the f6 doublings; f1 pair last gates
            # only its own feature matmuls.
            act(ms3, mp, 3 * W1)
            act(msh3, mp, 1.5 * W1)
            act(ms1, mp, W1)
            act(mc1, mp, W1, cos=True)
            tt(t6m[:], ms3[:], ms3[:], MULT)
            tt(mth3[:], msh3[:], msh3[:], MULT)
            ts(mc3[:], mth3[:], -2.0, 1.0, MULT, ADD)
            tt(sg6m[:], ms3[:], mc3[:], MULT)

            # ---- feature matmuls: alT[(nc)][np, m] -----------------------
            pairs = [
                (A["3c"], ms3), (A["6s"], t6m),
                (A["3s"], mc3), (A["6c"], sg6m),
                (A["1c"], ms1), (A["1s"], mc1),
            ]
            # All feature matmuls accumulate onto the zero-cleared alT with
            # start=False: order-independent, so the pair-outer emission
            # (pairs stream in plane-readiness order) is safe. With
            # start=True groups instead, interleaved-group reordering
            # clobbered accumulated values (0.19 rel err on device).
            np_ = len(pairs)
            for pi, (Aq, Bm) in enumerate(pairs):
                for nch in range(4):
                    for c in range(4):
                        nc.tensor.matmul(
                            alT[:, nch * 128:(nch + 1) * 128],
                            Bm[:, c * 512 + nch * 128: c * 512 + (nch + 1) * 128],
                            Aq[:, c * 128:(c + 1) * 128],
                            start=False,
                            stop=(pi == np_ - 1 and c == 3),
                        )

            # ---- softmax + output (transpose-free) -----------------------
            nc.scalar.activation(expT_sb[:], alT[:], AF.Exp)
            for nch in range(4):
                nc.tensor.matmul(
                    out_ps0[:], expT_sb[:, nch * 128:(nch + 1) * 128],
                    memN_sb[:, nch * 512: nch * 512 + 256],
                    start=False, stop=(nch == 3),
                )
                nc.tensor.matmul(
                    out_ps1[:], expT_sb[:, nch * 128:(nch + 1) * 128],
                    memN_sb[:, nch * 512 + 256: nch * 512 + 512],
                    start=False, stop=(nch == 3),
                )
                nc.tensor.matmul(
                    sums_ps[:], expT_sb[:, nch * 128:(nch + 1) * 128],
                    ones_sb[:],
                    start=(nch == 0), stop=(nch == 3),
                )
            # normalization happens on the host: ship the raw numerator and
            # row sums (saves recip + scale serialization on the tail).
            # PSUM -> SBUF copies split across ACT and DVE in parallel.
            dsl0, dsl1 = slice(0, 256), slice(256, 512)
            nc.scalar.activation(out_sb[:, dsl0], out_ps0[:], AF.Copy)
            nc.vector.tensor_copy(out_sb[:, dsl1], out_ps1[:])
            nc.vector.tensor_copy(sums_sb[:], sums_ps[:])
            nc.sync.dma_start(out.ap(), out_sb[:])
            nc.sync.dma_start(sums.ap(), sums_sb[:])

    nc.compile()
    return nc


_nc_cache = {}


def _get_nc():
    if "nc" not in _nc_cache:
        _nc_cache["nc"] = _build()
    return _nc_cache["nc"]


def _shard_inputs(query, memory, Wq, Wm, w_out):
    import ml_dtypes

    bf = ml_dtypes.bfloat16
    query = np.ascontiguousarray(query, dtype=np.float32)
    memory = np.ascontiguousarray(memory, dtype=np.float32)
    Wq = np.ascontiguousarray(Wq, dtype=np.float32)
    Wm = np.ascontiguousarray(Wm, dtype=np.float32)
    w_out = np.ascontiguousarray(w_out, dtype=np.float32)

    # fold sign of w into Wq/Wm rows (tanh odd), sort h by |w|
    sgn = np.sign(w_out)
    sgn[sgn == 0] = 1.0
    order = np.argsort(w_out * sgn)
    wtld = (w_out * sgn)[order]  # >= 0, [H]
    Wqp = (Wq * sgn[:, None])[order]
    Wmp = (Wm * sgn[:, None])[order]

    # [dp, (dc, c, hp)]
    wqT_h = np.ascontiguousarray(
        Wqp.T.reshape(4, 128, 4, 128).transpose(1, 0, 2, 3).reshape(128, 2048)
    ).astype(bf)
    wmT_h = np.ascontiguousarray(
        Wmp.T.reshape(4, 128, 4, 128).transpose(1, 0, 2, 3).reshape(128, 2048)
    ).astype(bf)

    # masks [hp, (u, c)]: mask_u[c*128+hp]
    msk_h = np.empty((128, len(MASKS) * 4), np.float32)
    for u, (_, alpha) in enumerate(MASKS):
        msk_h[:, u * 4:(u + 1) * 4] = (alpha * wtld).reshape(4, 128).T

    in_maps = []
    for i in range(NCORES):
        b, mh = divmod(i, 2)
        qT_h = np.ascontiguousarray(
            query[b, mh * ML:(mh + 1) * ML, :]
            .T.reshape(4, 128, 128).transpose(1, 0, 2).reshape(128, 512)
        ).astype(bf)
        memT_h = np.ascontiguousarray(
            memory[b].T.reshape(4, 128, 512).transpose(1, 0, 2).reshape(128, 2048)
        ).astype(bf)
        memN_h = np.ascontiguousarray(
            memory[b].reshape(4, 128, 512).transpose(1, 0, 2).reshape(128, 2048)
        ).astype(bf)
        in_maps.append({
            "qT": qT_h, "wqT": wqT_h, "wmT": wmT_h,
            "memT": memT_h, "memN": memN_h, "msk": msk_h,
        })
    return in_maps


def kernel(query, memory, Wq, Wm, w_out):
    nc = _get_nc()
    in_maps = _shard_inputs(query, memory, Wq, Wm, w_out)
    res = run_bass_kernel_spmd(nc, in_maps, core_ids=list(range(NCORES)))
    full = np.empty((B, M, D), dtype=np.float32)
    for i in range(NCORES):
        b, mh = divmod(i, 2)
        o = res.results[i]["out"].astype(np.float32)
        s = res.results[i]["sums"].astype(np.float32)
        full[b, mh * ML:(mh + 1) * ML, :] = o / s
    return full


# revision 3
# speedup vs baseline: 1.0354x; 1.0354x over previous
"""Bahdanau (additive MLP) attention on 8 Trainium2 NeuronCores — v3.

Approximation: tanh(x) ~= a1 sin(w1 x) + a3 sin(3 w1 x) + a6 sin(6 w1 x),
w1 = pi/L, coefficients polished against an exact bf16 plane-algebra
emulation of this kernel (end-to-end rel err 0.0121 vs the f32 reference).

aligns[m,n] = sum_h w~[h] tanh(qp[m,h] + mp[n,h]) expands via
sin(f(q+m)) = sin_f(q)cos_f(m) + cos_f(q)sin_f(m) into 6 PE pair-matmuls
with contraction (h). f6 planes derive from f3 by angle doubling:
  q side: sg6q = s3q*c3q (=sin6/2), t6q = s3q^2, c6q = 1-2 t6q (true cos)
  m side: t6m = s3m^2 (=(1-cos6)/2), sg6m = s3m*c3m (=sin6/2)
  a6[s6q c6m + c6q s6m] = 2 a6 sg6q (per-m const, drops in softmax)
                          - 4 a6 sg6q t6m + 2 a6 c6q sg6m
Pairs (q-plane folded with alpha*w~ masks, m-plane raw):
  (a1 w~ s1q, c1m) (a1 w~ c1q, s1m) (a3 w~ s3q, c3m) (a3 w~ c3q, s3m)
  (-4 a6 w~ sg6q, t6m) (2 a6 w~ c6q, sg6m)

Aligns are accumulated TRANSPOSED: alT[n-chunk][np, m] via stationary
m-plane slices, so softmax rowsums and the output matmul contract n on
partitions with no score transposes: sums = expT.T @ ones, out =
expT.T @ memN, both plain PE matmuls. exp runs per 128-wide n-chunk so
the output matmul pipelines behind it.

Schedule: DMAs ordered qT, wqT, wm/mem interleaved halves (single DMA
engine in the cost model is serial; q tensors first so the four q-side
sins pack the ACT engine while m-proj finishes), then 4 m-side sins
(f3 pair first: their DVE doublings feed feature matmuls immediately;
f1 pair last gates only its own 32 matmuls). No PE warmup/fillers: the
TimelineSim p-state ramp depends only on elapsed time, and all matmuls
here start after the 3us warm threshold.

Sharding: core i = batch i//2, M-half i%2 (128 query rows). Fully data
parallel, softmax local, no collectives.
"""

import numpy as np

import concourse.tile as tile
from concourse import bacc, mybir
from concourse.alu_op_type import AluOpType
from concourse.bass_utils import run_bass_kernel_spmd

f32 = mybir.dt.float32
bf16 = mybir.dt.bfloat16
AF = mybir.ActivationFunctionType
MULT = AluOpType.mult
ADD = AluOpType.add

B, M, N, D, H = 4, 256, 512, 512, 512
NCORES = 8
ML = M * B // NCORES  # 128 query rows per core

# tanh(x) ~= sum a_f sin(f*w1*x), f in {1,3,6}; polished fit (see docstring)
A1, A3, A6 = 1.17491041, 0.31016341, 0.08701841
L_RANGE = 8.58851971
W1 = np.pi / L_RANGE

# q-side fold masks: mask_u[h] = alpha_u * w~[h]
# f3/f6 m-side cosines are expanded through mth3 = sin^2(1.5 w1 mp):
#   c3m = 1-2 mth3, s6m = 2 ms3 - 4 ms3*mth3; per-m constants drop in
#   softmax, each remaining product becomes its own PE pair.
MASKS = (("1", A1), ("3c", A3), ("3s", -2 * A3),
         ("6s", -4 * A6), ("6c", 2 * A6), ("6c2", -4 * A6))
MIDX = {name: i for i, (name, _) in enumerate(MASKS)}


def _build():
    nc = bacc.Bacc("TRN2", target_bir_lowering=False, debug=False, num_devices=NCORES)

    qT = nc.dram_tensor("qT", [128, 512], bf16, kind="ExternalInput")
    wqT = nc.dram_tensor("wqT", [128, 2048], bf16, kind="ExternalInput")
    wmT = nc.dram_tensor("wmT", [128, 2048], bf16, kind="ExternalInput")
    memT = nc.dram_tensor("memT", [128, 2048], bf16, kind="ExternalInput")
    memN = nc.dram_tensor("memN", [128, 2048], bf16, kind="ExternalInput")
    msk = nc.dram_tensor("msk", [128, len(MASKS) * 4], f32, kind="ExternalInput")
    # col 512 carries the softmax row-sum (bf16) so one DMA ships both
    out = nc.dram_tensor("out", [128, 513], bf16, kind="ExternalOutput")

    with tile.TileContext(nc) as tc:
        with (
            tc.tile_pool(name="const", bufs=1) as const,
            tc.tile_pool(name="mpp", bufs=1, space="PSUM") as mpp,
            tc.tile_pool(name="qp_pool", bufs=1, space="PSUM") as qp_pool,
            tc.tile_pool(name="al_pool", bufs=1, space="PSUM") as al_pool,
        ):
            # ---- SBUF tiles ----------------------------------------------
            warm_sb = const.tile([128, 128], bf16)
            zeros_sb = const.tile([128, 512], bf16)
            qT_sb = const.tile([128, 512], bf16)
            wqT_sb = const.tile([128, 2048], bf16)
            wmT_sb = const.tile([128, 2048], bf16)
            memT_sb = const.tile([128, 2048], bf16)
            memN_sb = const.tile([128, 2048], bf16)
            msk_sb = const.tile([128, len(MASKS) * 4], f32)
            ones_sb = const.tile([128, 1], bf16)
            halfpi_sb = const.tile([128, 1], f32)

            qs1 = const.tile([128, 512], bf16, name="qs1")
            qc1 = const.tile([128, 512], bf16, name="qc1")
            qs3 = const.tile([128, 512], bf16, name="qs3")
            qc3 = const.tile([128, 512], bf16, name="qc3")
            qsh3 = const.tile([128, 512], bf16, name="qsh3")
            qth3 = const.tile([128, 512], bf16, name="qth3")
            t6q = const.tile([128, 512], bf16, name="t6q")
            sg6q = const.tile([128, 512], bf16, name="sg6q")
            c6q = const.tile([128, 512], bf16, name="c6q")
            ms3 = const.tile([128, 2048], bf16, name="ms3")
            msh3 = const.tile([128, 2048], bf16, name="msh3")
            mth3 = const.tile([128, 2048], bf16, name="mth3")
            w6m = const.tile([128, 2048], bf16, name="w6m")
            ms1 = const.tile([128, 2048], bf16, name="ms1")
            mc1 = const.tile([128, 2048], bf16, name="mc1")
            t6m = const.tile([128, 2048], bf16, name="t6m")
            A = {k: const.tile([128, 512], bf16, name=f"A{k}")
                 for k in ("1s", "1c", "3s", "3c", "6s", "6c", "6c2")}
            expT_sb = const.tile([128, 512], bf16, name="expT")
            out_sb = const.tile([128, 513], bf16)

            # ---- PSUM tiles ----------------------------------------------
            mp = mpp.tile([128, 2048], f32, tag="mp", name="mp")
            qp = qp_pool.tile([128, 512], f32, tag="qp", name="qp")
            alT = al_pool.tile([128, 512], f32, tag="al", name="alT")
            # two separate output PSUM tiles so the ACT and DVE copies at
            # the tail overlap (one shared tile serializes them)
            out_ps0 = qp_pool.tile([128, 256], f32, tag="op0", name="out_ps0")
            out_ps1 = qp_pool.tile([128, 256], f32, tag="op1", name="out_ps1")
            # reuses qp's bank (qp is dead after the q-side activations)
            sums_ps = qp_pool.tile([128, 1], f32, tag="qp", name="sums_ps")

            # ---- prologue ------------------------------------------------
            nc.vector.memset(ones_sb[:], 1.0)
            nc.vector.memset(halfpi_sb[:], float(np.pi / 2))
            nc.vector.memset(warm_sb[:], 1.0)
            nc.vector.memset(zeros_sb[:], 0.0)

            # DMA order (single serial DMA engine in the cost model):
            # q tensors first, then wm/mem interleaved halves, memN last.
            # big wqT first: q-proj is gated by max(qT, wqT) arrival, so the
            # small qT rides behind without delaying it
            nc.sync.dma_start(wqT_sb[:], wqT.ap())
            nc.sync.dma_start(qT_sb[:], qT.ap())
            # wm/mem slices sized (1024,512,512): the big slice lands while
            # q-proj runs; the small trailing slices keep the last m-proj
            # group's start early without dispatch-queue serialization.
            for sl in (slice(0, 1024), slice(1024, 1536), slice(1536, 2048)):
                nc.sync.dma_start(wmT_sb[:, sl], wmT.ap()[:, sl])
                nc.sync.dma_start(memT_sb[:, sl], memT.ap()[:, sl])
            nc.sync.dma_start(memN_sb[:], memN.ap())
            nc.gpsimd.dma_start(msk_sb[:], msk.ap())

            # PE warmups: TimelineSim's p-state clock starts at the first
            # matmul; issue cheap ones immediately so the real matmuls
            # (first at ~4.7us) run at full rate. out_ps is overwritten
            # later by an accumulation group with start=True.
            # PE warmups that double as zero-clears of the accumulation
            # targets (writes 0 and sets has_written), so every later
            # matmul accumulates with start=False in ANY schedule order --
            # interleaved start=True groups get clobbered when the
            # scheduler reorders them. All warmups write zeros, so their
            # order relative to each other is irrelevant too.
            for _ in range(3):
                nc.tensor.matmul(alT[:], warm_sb[:], zeros_sb[:],
                                 start=True, stop=True)
            nc.tensor.matmul(out_ps0[:], warm_sb[:], zeros_sb[:, 0:256],
                             start=True, stop=True)
            nc.tensor.matmul(out_ps1[:], warm_sb[:], zeros_sb[:, 0:256],
                             start=True, stop=True)
            # NOTE: no zero-clear for sums_ps -- it aliases qp's bank, and
            # an early PE-FIFO clear waiting on qp's death deadlocks the
            # queue. Its 4-matmul group below keeps start=True instead
            # (contiguous, uniform readiness).

            # ---- projections ---------------------------------------------
            # q-proj: qp[hp, (c,m)] = Wq~.T @ query.T   (c-outer, dc-inner)
            for c in range(4):
                for dc in range(4):
                    nc.tensor.matmul(
                        qp[:, c * 128:(c + 1) * 128],
                        wqT_sb[:, dc * 512 + c * 128: dc * 512 + (c + 1) * 128],
                        qT_sb[:, dc * 128:(dc + 1) * 128],
                        start=(dc == 0), stop=(dc == 3),
                    )
            # m-proj: mp[hp, (c,n)] (dc-outer so dc-halves consume the
            # wm/mem DMA halves as they land)
            for dc in range(4):
                for c in range(4):
                    nc.tensor.matmul(
                        mp[:, c * 512:(c + 1) * 512],
                        wmT_sb[:, dc * 512 + c * 128: dc * 512 + (c + 1) * 128],
                        memT_sb[:, dc * 512:(dc + 1) * 512],
                        start=(dc == 0), stop=(dc == 3),
                    )

            # ---- q-side planes + folds -----------------------------------
            def act(dst, src, w, cos=False):
                if cos:
                    nc.scalar.activation(dst[:], src[:], AF.Sin,
                                         bias=halfpi_sb[:], scale=float(w))
                else:
                    nc.scalar.activation(dst[:], src[:], AF.Sin, scale=float(w))

            # f3 cos via half-angle: cos(3w1 x) = 1 - 2 sin^2(1.5 w1 x).
            # The direct form sin(3w1 x + pi/2) sends arguments past 2*pi
            # where the device Sin table degrades (the sin planes stay under
            # 2*pi and are computed directly).
            act(qs1, qp, W1)
            act(qc1, qp, W1, cos=True)
            act(qs3, qp, 3 * W1)
            act(qsh3, qp, 1.5 * W1)

            tt = nc.vector.tensor_tensor
            ts = nc.vector.tensor_scalar

            tt(qth3[:], qsh3[:], qsh3[:], MULT)
            ts(qc3[:], qth3[:], -2.0, 1.0, MULT, ADD)
            tt(t6q[:], qs3[:], qs3[:], MULT)
            tt(sg6q[:], qs3[:], qc3[:], MULT)
            ts(c6q[:], t6q[:], -2.0, 1.0, MULT, ADD)

            def fold(dst, srcp, name, eng):
                u = MIDX[name]
                for c in range(4):
                    eng.tensor_scalar_mul(
                        dst[:, c * 128:(c + 1) * 128],
                        srcp[:, c * 128:(c + 1) * 128],
                        msk_sb[:, u * 4 + c: u * 4 + c + 1],
                    )

            # f1 folds on gpsimd (idle engine), rest on DVE
            fold(A["1s"], qs1, "1", nc.gpsimd)
            fold(A["1c"], qc1, "1", nc.gpsimd)
            fold(A["3s"], qs3, "3s", nc.vector)
            fold(A["3c"], qc3, "3c", nc.vector)
            fold(A["6s"], sg6q, "6s", nc.vector)
            fold(A["6c"], c6q, "6c", nc.vector)
            fold(A["6c2"], c6q, "6c2", nc.gpsimd)

            # ---- m-side planes -------------------------------------------
            # f3 pair first: feeds the f6 doublings; f1 pair last gates
            # only its own feature matmuls.
            act(ms3, mp, 3 * W1)
            act(msh3, mp, 1.5 * W1)
            act(ms1, mp, W1)
            act(mc1, mp, W1, cos=True)
            tt(t6m[:], ms3[:], ms3[:], MULT)
            tt(mth3[:], msh3[:], msh3[:], MULT)
            tt(w6m[:], ms3[:], mth3[:], MULT)

            # ---- feature matmuls: alT[(nc)][np, m] -----------------------
            # emission in m-plane readiness order (PE queue is FIFO)
            pairs = [
                (A["3c"], ms3), (A["6c"], ms3), (A["6s"], t6m),
                (A["3s"], mth3), (A["1c"], ms1), (A["6c2"], w6m),
                (A["1s"], mc1),
            ]
            # All feature matmuls accumulate onto the zero-cleared alT with
            # start=False: order-independent, so the pair-outer emission
            # (pairs stream in plane-readiness order) is safe. With
            # start=True groups instead, interleaved-group reordering
            # clobbered accumulated values (0.19 rel err on device).
            np_ = len(pairs)
            for pi, (Aq, Bm) in enumerate(pairs):
                for nch in range(4):
                    for c in range(4):
                        nc.tensor.matmul(
                            alT[:, nch * 128:(nch + 1) * 128],
                            Bm[:, c * 512 + nch * 128: c * 512 + (nch + 1) * 128],
                            Aq[:, c * 128:(c + 1) * 128],
                            start=False,
                            stop=(pi == np_ - 1 and c == 3),
                        )

            # ---- softmax + output (transpose-free) -----------------------
            # exp in two halves so the first output matmuls overlap the
            # second half
            nc.scalar.activation(expT_sb[:, 0:256], alT[:, 0:256], AF.Exp)
            nc.scalar.activation(expT_sb[:, 256:512], alT[:, 256:512], AF.Exp)
            for nch in range(4):
                nc.tensor.matmul(
                    out_ps0[:], expT_sb[:, nch * 128:(nch + 1) * 128],
                    memN_sb[:, nch * 512: nch * 512 + 256],
                    start=False, stop=(nch == 3),
                )
                nc.tensor.matmul(
                    out_ps1[:], expT_sb[:, nch * 128:(nch + 1) * 128],
                    memN_sb[:, nch * 512 + 256: nch * 512 + 512],
                    start=False, stop=(nch == 3),
                )
                nc.tensor.matmul(
                    sums_ps[:], expT_sb[:, nch * 128:(nch + 1) * 128],
                    ones_sb[:],
                    start=(nch == 0), stop=(nch == 3),
                )
            # normalization happens on the host: ship the raw numerator and
            # row sums (saves recip + scale serialization on the tail).
            # PSUM -> SBUF copies split across ACT and DVE in parallel.
            dsl0, dsl1 = slice(0, 256), slice(256, 512)
            nc.scalar.activation(out_sb[:, dsl0], out_ps0[:], AF.Copy)
            nc.vector.tensor_copy(out_sb[:, dsl1], out_ps1[:])
            nc.vector.tensor_copy(out_sb[:, 512:513], sums_ps[:])
            nc.sync.dma_start(out.ap(), out_sb[:])

    nc.compile()
    return nc


_nc_cache = {}


def _get_nc():
    if "nc" not in _nc_cache:
        _nc_cache["nc"] = _build()
    return _nc_cache["nc"]


def _shard_inputs(query, memory, Wq, Wm, w_out):
    import ml_dtypes

    bf = ml_dtypes.bfloat16
    query = np.ascontiguousarray(query, dtype=np.float32)
    memory = np.ascontiguousarray(memory, dtype=np.float32)
    Wq = np.ascontiguousarray(Wq, dtype=np.float32)
    Wm = np.ascontiguousarray(Wm, dtype=np.float32)
    w_out = np.ascontiguousarray(w_out, dtype=np.float32)

    # fold sign of w into Wq/Wm rows (tanh odd), sort h by |w|
    sgn = np.sign(w_out)
    sgn[sgn == 0] = 1.0
    order = np.argsort(w_out * sgn)
    wtld = (w_out * sgn)[order]  # >= 0, [H]
    Wqp = (Wq * sgn[:, None])[order]
    Wmp = (Wm * sgn[:, None])[order]

    # [dp, (dc, c, hp)]
    wqT_h = np.ascontiguousarray(
        Wqp.T.reshape(4, 128, 4, 128).transpose(1, 0, 2, 3).reshape(128, 2048)
    ).astype(bf)
    wmT_h = np.ascontiguousarray(
        Wmp.T.reshape(4, 128, 4, 128).transpose(1, 0, 2, 3).reshape(128, 2048)
    ).astype(bf)

    # masks [hp, (u, c)]: mask_u[c*128+hp]
    msk_h = np.empty((128, len(MASKS) * 4), np.float32)
    for u, (_, alpha) in enumerate(MASKS):
        msk_h[:, u * 4:(u + 1) * 4] = (alpha * wtld).reshape(4, 128).T

    in_maps = []
    for i in range(NCORES):
        b, mh = divmod(i, 2)
        qT_h = np.ascontiguousarray(
            query[b, mh * ML:(mh + 1) * ML, :]
            .T.reshape(4, 128, 128).transpose(1, 0, 2).reshape(128, 512)
        ).astype(bf)
        memT_h = np.ascontiguousarray(
            memory[b].T.reshape(4, 128, 512).transpose(1, 0, 2).reshape(128, 2048)
        ).astype(bf)
        memN_h = np.ascontiguousarray(
            memory[b].reshape(4, 128, 512).transpose(1, 0, 2).reshape(128, 2048)
        ).astype(bf)
        in_maps.append({
            "qT": qT_h, "wqT": wqT_h, "wmT": wmT_h,
            "memT": memT_h, "memN": memN_h, "msk": msk_h,
        })
    return in_maps


def kernel(query, memory, Wq, Wm, w_out):
    nc = _get_nc()
    in_maps = _shard_inputs(query, memory, Wq, Wm, w_out)
    res = run_bass_kernel_spmd(nc, in_maps, core_ids=list(range(NCORES)))
    full = np.empty((B, M, D), dtype=np.float32)
    for i in range(NCORES):
        b, mh = divmod(i, 2)
        o = res.results[i]["out"].astype(np.float32)
        full[b, mh * ML:(mh + 1) * ML, :] = o[:, 0:512] / o[:, 512:513]
    return full
